# revision 12
# baseline (speedup 1.0000x reference)
"""Trainium2 Bass kernel for a 2-layer LSTM (B=256, T=512, D=64, H=512) + FC on last step.

Sharding: data-parallel over batch — 32 samples per NeuronCore on 8 cores.
Per-core design (everything SBUF-resident, no HBM traffic inside the loop):
  - gates = state.T @ W on PE with the (small, M=32) state stationary and the
    weights streaming in N=512 chunks at full rate.
  - V3: all 4 gate chunks of a layer-step land in ONE PSUM bank at partition
    offsets 32c (tile_position is inferred from out.base_partition), weights
    are column-permuted to gate order (i,f,o,g) so one [96,512] sigmoid and
    one [32,512] tanh cover a whole step; nonlinearities/cell math run in
    bf16 (DVE 2-4x modes); layer 1 is software-pipelined one 32-step block
    behind layer 0 so each layer's ACT/DVE chain hides under the other
    layer's PE matmul burst.
  - layer-1 input projection is precomputed per 32-step block as a bulk GEMM
    at M=128 (4 steps per GEMM), biases folded via a ones-row; the per-step
    injection goes through a K=32 identity matmul at matching row/col offsets.
"""

import numpy as np
import ml_dtypes

import concourse.bass as bass
import concourse.mybir as mybir
import concourse.tile as tile
from concourse.bass_utils import run_bass_kernel_spmd

BF16 = mybir.dt.bfloat16
F32 = mybir.dt.float32

B, T, D, H, O = 256, 512, 64, 512, 1
G = 4 * H  # 2048
NCORES = 8
BL = B // NCORES  # 32
NK_H = H // 128  # 4 K-chunks for an H-sized contraction
NN = G // 512  # 4 N-chunks of 512 gate columns
SIG = mybir.ActivationFunctionType.Sigmoid
COPY = mybir.ActivationFunctionType.Copy
TANH = mybir.ActivationFunctionType.Tanh
SB = 32  # steps per layer-1 pipeline block


def _split_excess_waits(nc, max_waits: int = 1) -> int:
    """This container's walrus rejects >1 sync wait per instruction; move
    excess waits onto preceding same-engine NOPs (same-engine earlier wait
    is ordering-equivalent)."""
    n_split = 0
    for f in nc.m.functions:
        for bb in f.blocks:
            new_insts = []
            for inst in bb.instructions:
                si = inst.sync_info
                if si is not None and si.on_wait and len(si.on_wait) > max_waits:
                    waits = list(si.on_wait)
                    while len(waits) > max_waits:
                        chunk, waits = waits[:max_waits], waits[max_waits:]
                        nop = mybir.InstNoOp(
                            name=f"{inst.name}-wsplit-{n_split}", ins=[], outs=[]
                        )
                        nop.engine = inst.engine
                        nop.sync_info = mybir.SyncInfo(on_wait=chunk, on_update=[])
                        new_insts.append(nop)
                        n_split += 1
                    si.on_wait = waits
                new_insts.append(inst)
            bb.instructions[:] = new_insts
    return n_split


def build_lstm_nc(t_steps: int = T):
    assert t_steps % SB == 0 and SB % 4 == 0
    nblk = t_steps // SB
    nc = bass.Bass("TRN2")

    xt_d = nc.dram_tensor("xt", [D + 1, t_steps, BL], BF16, kind="ExternalInput")
    w0a_d = nc.dram_tensor("w0a", [D + 1, G], BF16, kind="ExternalInput")
    w0b_d = nc.dram_tensor("w0b", [128, NK_H, G], BF16, kind="ExternalInput")
    w1_d = nc.dram_tensor("w1", [128, 2 * NK_H, G], BF16, kind="ExternalInput")
    w1bias_d = nc.dram_tensor("w1bias", [1, G], BF16, kind="ExternalInput")
    fcw_d = nc.dram_tensor("fcw", [128, NK_H], BF16, kind="ExternalInput")
    ident4_d = nc.dram_tensor("ident4", [128, BL], BF16, kind="ExternalInput")
    fcb_d = nc.dram_tensor("fcb", [1, 1], F32, kind="ExternalInput")
    y_d = nc.dram_tensor("y", [BL, O], F32, kind="ExternalOutput")

    with tile.TileContext(nc) as tc:
        with (
            tc.tile_pool(name="singles", bufs=1) as singles,
            tc.tile_pool(name="state", bufs=1) as state,
            tc.tile_pool(name="work", bufs=3) as work,
            tc.tile_pool(name="gps", bufs=2, space="PSUM") as gps,
            tc.tile_pool(name="tpps", bufs=2, space="PSUM") as tpps,
            tc.tile_pool(name="bkps", bufs=2, space="PSUM") as bkps,
        ):
            # --- resident constants ---
            xt_s = singles.tile([D + 1, t_steps, BL], BF16)
            nc.sync.dma_start(out=xt_s, in_=xt_d[:, :, :])
            w0a_s = singles.tile([D + 1, G], BF16)
            nc.sync.dma_start(out=w0a_s, in_=w0a_d[:, :])
            w0b_s = singles.tile([128, NK_H, G], BF16)
            nc.sync.dma_start(out=w0b_s, in_=w0b_d[:, :, :])
            w1_s = singles.tile([128, 2 * NK_H, G], BF16)
            nc.sync.dma_start(out=w1_s, in_=w1_d[:, :, :])
            w1b_s = singles.tile([1, G], BF16)
            nc.sync.dma_start(out=w1b_s, in_=w1bias_d[:, :])
            fcw_s = singles.tile([128, NK_H], BF16)
            nc.sync.dma_start(out=fcw_s, in_=fcw_d[:, :])
            fcb_s = singles.tile([BL, 1], F32)
            nc.sync.dma_start(out=fcb_s, in_=fcb_d[:, :].to_broadcast((BL, 1)))
            ones_r128 = singles.tile([1, 128], BF16)
            nc.vector.memset(ones_r128, 1.0)
            ident4_s = singles.tile([128, BL], BF16)
            nc.sync.dma_start(out=ident4_s, in_=ident4_d[:, :])

            # --- recurrent state ---
            h0blk = state.tile([128, NK_H, SB, BL], BF16)  # layer0 h history
            h1T = state.tile([128, NK_H, BL], BF16)
            c0 = state.tile([128, 128], BF16)
            c1 = state.tile([128, 128], BF16)
            xp1blk_a = state.tile([128, SB // 4, NN, 512], BF16)
            xp1blk_b = state.tile([128, SB // 4, NN, 512], BF16)
            xp1blk = [xp1blk_a, xp1blk_b]
            for st in (h0blk, h1T, c0, c1):
                nc.vector.memset(st, 0.0)

            def gates_mm(first_k, w_s, kslices):
                """All NN gate chunks of one step into ONE psum bank at
                partition offsets 32c (explicit tile_position: the auto path
                rejects base partition 96). first_k(n) -> (lhsT, rhs, row)
                for the leading accumulation chunk of column-chunk n."""
                bank = gps.tile([128, 512], F32, tag="g")
                # all state-independent leading matmuls first: they give the
                # PE ~850ns of work while the hT transpose->copy->sem chain
                # of the previous step drains.
                for n in range(NN):
                    lhsT0, rhs0, row0 = first_k(n)
                    nc.tensor.matmul(
                        bank[n * BL : (n + 1) * BL, :], lhsT0, rhs0,
                        start=True, stop=False,
                        tile_position=(row0, n * BL),
                    )
                for n in range(NN):
                    out = bank[n * BL : (n + 1) * BL, :]
                    for j, (lhsT_k, wk) in enumerate(kslices):
                        nc.tensor.matmul(
                            out,
                            lhsT_k,
                            w_s[:, wk, n * 512 : (n + 1) * 512],
                            start=False,
                            stop=(j == len(kslices) - 1),
                            tile_position=(0, n * BL),
                        )
                return bank

            def chain(bank, cell, tag):
                """Nonlinearities + cell update, all bf16, all ops on
                [128,*] same-partition tiles. Bank partition block c holds
                hidden units [128c:128c+128); free layout per chunk is
                [i|f|o|g] x 128 (host-side column permutation). cell/h are
                [128=(c,b), 128=j] with j the unit within block c."""
                sig_ifo = work.tile([128, 384], BF16, tag=f"sig{tag}")
                nc.scalar.activation(sig_ifo, bank[:, 0:384], SIG)
                tanh_g = work.tile([128, 128], BF16, tag=f"tg{tag}")
                nc.scalar.activation(tanh_g, bank[:, 384:512], TANH)
                ig = work.tile([128, 128], BF16, tag=f"ig{tag}")
                nc.gpsimd.tensor_mul(ig, sig_ifo[:, 0:128], tanh_g)
                nc.gpsimd.tensor_mul(cell, cell, sig_ifo[:, 128:256])
                nc.gpsimd.tensor_add(cell, cell, ig)
                tanh_c = work.tile([128, 128], BF16, tag=f"tc{tag}")
                nc.scalar.activation(tanh_c, cell, TANH)
                h_new = work.tile([128, 128], BF16, tag=f"hn{tag}")
                nc.gpsimd.tensor_mul(h_new, sig_ifo[:, 256:384], tanh_c)
                return h_new

            def transpose_to(h_new, dst):
                """h_new [128=(c,b), 128=j] -> dst [128, NK_H, 32] (hT).
                Per-block transpose [32,128]->[128,32]; per-chunk copies so
                the consumer's first LDW waits only on its own chunk."""
                tp = tpps.tile([128, NK_H, BL], BF16, tag="tp")
                for k in range(NK_H):
                    nc.tensor.transpose(
                        tp[:, k, :],
                        h_new[k * BL : (k + 1) * BL, :],
                        ident4_s[k * BL : (k + 1) * BL, :],
                        tile_position=(k * BL, 0),
                    )
                    nc.vector.tensor_copy(dst[:, k, :], tp[:, k, :])

            h0_new = None  # h_new of layer0 step t-1 (awaiting transpose)
            h1_new = None  # h_new of layer1 step t'-1

            # Pipeline: iteration (b, s) runs layer0 step t=b*SB+s and
            # layer1 step t-SB. Issue order per pair keeps PE fed:
            #   tp0(t-1) -> L0mm(t) -> tp1(t'-1) -> L1mm(t') -> chains.
            # h0blk slot j holds h0 of step b*SB+j (written at iter s=j+1,
            # slot (s-1)%SB). The bulk GEMM for a 4-step group fires at the
            # next s%4==0 boundary, once all 4 slots are written.
            for b in range(nblk + 1):
                for s in range(SB):
                    t = b * SB + s
                    if h0_new is not None and (b < nblk or s == 0):
                        transpose_to(h0_new, h0blk[:, :, (s - 1) % SB, :])
                        h0_new = None
                    if b < nblk:
                        prev = h0blk[:, :, (s - 1) % SB, :]
                        g0 = gates_mm(
                            lambda n, t=t: (
                                xt_s[:, t, :],
                                w0a_s[:, n * 512 : (n + 1) * 512],
                                0,
                            ),
                            w0b_s,
                            [(prev[:, k, :], k) for k in range(NK_H)],
                        )
                    if b >= 1:
                        if h1_new is not None:
                            transpose_to(h1_new, h1T)
                            h1_new = None
                        xp = xp1blk[(b - 1) % 2]
                        g1 = gates_mm(
                            lambda n, s=s: (
                                ident4_s[(s % 4) * BL : (s % 4 + 1) * BL, :],
                                xp[(s % 4) * BL : (s % 4 + 1) * BL, s // 4, n, :],
                                (s % 4) * BL,
                            ),
                            w1_s,
                            [(h1T[:, k, :], NK_H + k) for k in range(NK_H)],
                        )
                    if b < nblk:
                        h0_new = chain(g0, c0, "0")
                    if b >= 1:
                        h1_new = chain(g1, c1, "1")
                    # bulk layer-1 input projection for the just-completed
                    # 4-step group (slots s-4..s-1 of block bb).
                    if s % 4 == 0 and t >= 4 and (b < nblk or s == 0):
                        bb, m = (b, s // 4 - 1) if s > 0 else (b - 1, SB // 4 - 1)
                        xpo = xp1blk[bb % 2]
                        for n in range(NN):
                            ns = slice(n * 512, (n + 1) * 512)
                            xps = bkps.tile([128, 512], F32, tag="bk")
                            nc.tensor.matmul(
                                xps, ones_r128, w1b_s[:, ns],
                                start=True, stop=False,
                            )
                            for k in range(NK_H):
                                nc.tensor.matmul(
                                    xps,
                                    h0blk[:, k, 4 * m : 4 * m + 4, :].rearrange(
                                        "p a b -> p (a b)"
                                    ),
                                    w1_s[:, k, ns],
                                    start=False,
                                    stop=(k == NK_H - 1),
                                )
                            nc.vector.tensor_copy(xpo[:, m, n, 0:256], xps[:, 0:256])
                            nc.vector.tensor_copy(xpo[:, m, n, 256:512], xps[:, 256:512])

            # flush the last transposes (layer1's final h for fc)
            if h1_new is not None:
                transpose_to(h1_new, h1T)

            # --- fc on last h1 ---
            fcp = bkps.tile([BL, O], F32, tag="fc")
            for k in range(NK_H):
                nc.tensor.matmul(
                    fcp,
                    h1T[:, k, :],
                    fcw_s[:, k : k + 1],
                    start=(k == 0),
                    stop=(k == NK_H - 1),
                )
            y_s = work.tile([BL, O], F32, tag="y")
            nc.vector.tensor_add(y_s, fcp, fcb_s)
            nc.sync.dma_start(out=y_d[:, :], in_=y_s)

    _split_excess_waits(nc)
    return nc


# host-side gate-column permutation: torch gate order is (i,f,g,o); chunk c
# holds [i|f|o|g] x 128 for hidden units [128c:128c+128) so that sigmoid
# covers free cols 0:384 and tanh 384:512 across all partition blocks.
_PERM = np.concatenate(
    [np.concatenate(
        [np.arange(g * H + 128 * c, g * H + 128 * c + 128) for g in (0, 1, 3, 2)]
    ) for c in range(NK_H)]
)


def prep_inputs(x, w_ih_0, w_hh_0, b_ih_0, b_hh_0, w_ih_1, w_hh_1, b_ih_1, b_hh_1,
                fc_w, fc_b, t_steps: int = T):
    """Host-side layout prep + sharding. Returns per-core in_maps."""
    bf = ml_dtypes.bfloat16
    w0a = np.concatenate(
        [w_ih_0.T, (b_ih_0 + b_hh_0)[None, :]], axis=0
    )[:, _PERM].astype(bf)  # [65, G]
    w0b = np.ascontiguousarray(
        w_hh_0.T[:, _PERM].reshape(NK_H, 128, G).transpose(1, 0, 2)
    ).astype(bf)  # [128, 4, G]
    w1 = np.ascontiguousarray(
        np.concatenate([w_ih_1.T, w_hh_1.T], axis=0)[:, _PERM]
        .reshape(2 * NK_H, 128, G)
        .transpose(1, 0, 2)
    ).astype(bf)  # [128, 8, G]
    w1bias = (b_ih_1 + b_hh_1)[None, _PERM].astype(bf)  # [1, G]
    fcw = np.ascontiguousarray(fc_w.reshape(NK_H, 128).T).astype(bf)  # [128, 4]
    fcb = fc_b.reshape(1, 1).astype(np.float32)
    ident4 = np.concatenate([np.eye(BL, dtype=np.float32)] * 4, axis=0).astype(bf)

    in_maps = []
    for c in range(NCORES):
        xc = x[c * BL : (c + 1) * BL, :t_steps, :]  # [32, T, 64]
        xt = np.transpose(xc, (2, 1, 0))  # [64, T, 32]
        xt = np.concatenate([xt, np.ones((1, t_steps, BL), np.float32)], axis=0)
        in_maps.append(
            {
                "xt": np.ascontiguousarray(xt).astype(bf),
                "w0a": w0a,
                "w0b": w0b,
                "w1": w1,
                "w1bias": w1bias,
                "fcw": fcw,
                "fcb": fcb,
                "ident4": ident4,
            }
        )
    return in_maps


_NC_CACHE = {}


def kernel(x, w_ih_0, w_hh_0, b_ih_0, b_hh_0, w_ih_1, w_hh_1, b_ih_1, b_hh_1,
           fc_w, fc_b):
    x = np.asarray(x, np.float32)
    args = [np.asarray(a, np.float32) for a in (
        w_ih_0, w_hh_0, b_ih_0, b_hh_0, w_ih_1, w_hh_1, b_ih_1, b_hh_1, fc_w, fc_b)]
    if T not in _NC_CACHE:
        _NC_CACHE[T] = build_lstm_nc(T)
    nc = _NC_CACHE[T]
    in_maps = prep_inputs(x, *args, t_steps=T)
    res = run_bass_kernel_spmd(nc, in_maps, core_ids=list(range(NCORES)))
    return np.concatenate([res.results[c]["y"] for c in range(NCORES)], axis=0)


# revision 17
# speedup vs baseline: 1.1669x; 1.1669x over previous
"""Trainium2 Bass kernel for a 2-layer LSTM (B=256, T=512, D=64, H=512) + FC on last step.

Sharding: data-parallel over batch — 32 samples per NeuronCore on 8 cores.
Per-core design (everything SBUF-resident, no HBM traffic inside the loop):
  - gates = state.T @ W on PE with the (small, M=32) state stationary and the
    weights streaming in N=512 chunks at full rate.
  - V3: all 4 gate chunks of a layer-step land in ONE PSUM bank at partition
    offsets 32c (tile_position is inferred from out.base_partition), weights
    are column-permuted to gate order (i,f,o,g) so one [96,512] sigmoid and
    one [32,512] tanh cover a whole step; nonlinearities/cell math run in
    bf16 (DVE 2-4x modes); layer 1 is software-pipelined one 32-step block
    behind layer 0 so each layer's ACT/DVE chain hides under the other
    layer's PE matmul burst.
  - layer-1 input projection is precomputed per 32-step block as a bulk GEMM
    at M=128 (4 steps per GEMM), biases folded via a ones-row; the per-step
    injection goes through a K=32 identity matmul at matching row/col offsets.
"""

import numpy as np
import ml_dtypes

import concourse.bass as bass
import concourse.mybir as mybir
import concourse.tile as tile
from concourse.bass_utils import run_bass_kernel_spmd

BF16 = mybir.dt.bfloat16
F32 = mybir.dt.float32

B, T, D, H, O = 256, 512, 64, 512, 1
G = 4 * H  # 2048
NCORES = 8
BL = B // NCORES  # 32
NK_H = H // 128  # 4 K-chunks for an H-sized contraction
NN = G // 512  # 4 N-chunks of 512 gate columns
SIG = mybir.ActivationFunctionType.Sigmoid
COPY = mybir.ActivationFunctionType.Copy
TANH = mybir.ActivationFunctionType.Tanh
SB = 32  # steps per layer-1 pipeline block


def _split_excess_waits(nc, max_waits: int = 1) -> int:
    """This container's walrus rejects >1 sync wait per instruction; move
    excess waits onto preceding same-engine NOPs (same-engine earlier wait
    is ordering-equivalent)."""
    n_split = 0
    for f in nc.m.functions:
        for bb in f.blocks:
            new_insts = []
            for inst in bb.instructions:
                si = inst.sync_info
                if si is not None and si.on_wait and len(si.on_wait) > max_waits:
                    waits = list(si.on_wait)
                    while len(waits) > max_waits:
                        chunk, waits = waits[:max_waits], waits[max_waits:]
                        nop = mybir.InstNoOp(
                            name=f"{inst.name}-wsplit-{n_split}", ins=[], outs=[]
                        )
                        nop.engine = inst.engine
                        nop.sync_info = mybir.SyncInfo(on_wait=chunk, on_update=[])
                        new_insts.append(nop)
                        n_split += 1
                    si.on_wait = waits
                new_insts.append(inst)
            bb.instructions[:] = new_insts
    return n_split


def build_lstm_nc(t_steps: int = T):
    assert t_steps % SB == 0 and SB % 4 == 0
    nblk = t_steps // SB
    nc = bass.Bass("TRN2")

    xt_d = nc.dram_tensor("xt", [D + 1, t_steps, BL], BF16, kind="ExternalInput")
    w0a_d = nc.dram_tensor("w0a", [D + 1, G], BF16, kind="ExternalInput")
    w0b_d = nc.dram_tensor("w0b", [128, NK_H, G], BF16, kind="ExternalInput")
    w1_d = nc.dram_tensor("w1", [128, 2 * NK_H, G], BF16, kind="ExternalInput")
    w1bias_d = nc.dram_tensor("w1bias", [1, G], BF16, kind="ExternalInput")
    fcw_d = nc.dram_tensor("fcw", [128, NK_H], BF16, kind="ExternalInput")
    ident128_d = nc.dram_tensor("ident128", [128, 128], BF16, kind="ExternalInput")
    fcb_d = nc.dram_tensor("fcb", [1, 1], F32, kind="ExternalInput")
    y_d = nc.dram_tensor("y", [BL, O], F32, kind="ExternalOutput")

    with tile.TileContext(nc) as tc:
        with (
            tc.tile_pool(name="singles", bufs=1) as singles,
            tc.tile_pool(name="state", bufs=1) as state,
            tc.tile_pool(name="work", bufs=3) as work,
            tc.tile_pool(name="gps", bufs=2, space="PSUM") as gps,
            tc.tile_pool(name="tpps", bufs=2, space="PSUM") as tpps,
            tc.tile_pool(name="bkps", bufs=2, space="PSUM") as bkps,
        ):
            # --- resident constants ---
            xt_s = singles.tile([D + 1, t_steps, BL], BF16)
            nc.sync.dma_start(out=xt_s, in_=xt_d[:, :, :])
            w0a_s = singles.tile([D + 1, G], BF16)
            nc.sync.dma_start(out=w0a_s, in_=w0a_d[:, :])
            w0b_s = singles.tile([128, NK_H, G], BF16)
            nc.sync.dma_start(out=w0b_s, in_=w0b_d[:, :, :])
            w1_s = singles.tile([128, 2 * NK_H, G], BF16)
            nc.sync.dma_start(out=w1_s, in_=w1_d[:, :, :])
            w1b_s = singles.tile([1, G], BF16)
            nc.sync.dma_start(out=w1b_s, in_=w1bias_d[:, :])
            fcw_s = singles.tile([128, NK_H], BF16)
            nc.sync.dma_start(out=fcw_s, in_=fcw_d[:, :])
            fcb_s = singles.tile([BL, 1], F32)
            nc.sync.dma_start(out=fcb_s, in_=fcb_d[:, :].to_broadcast((BL, 1)))
            ones_r128 = singles.tile([1, 128], BF16)
            nc.vector.memset(ones_r128, 1.0)
            ident128_s = singles.tile([128, 128], BF16)
            nc.sync.dma_start(out=ident128_s, in_=ident128_d[:, :])

            # --- recurrent state ---
            h0blk = state.tile([128, NK_H, SB, BL], BF16)  # layer0 h history
            h1T = state.tile([128, NK_H, BL], BF16)
            c0 = state.tile([128, 128], BF16)
            c1 = state.tile([128, 128], BF16)
            xp1R_a = state.tile([128, SB, 512], BF16)
            xp1R_b = state.tile([128, SB, 512], BF16)
            xp1R = [xp1R_a, xp1R_b]
            for st in (h0blk, h1T, c0, c1):
                nc.vector.memset(st, 0.0)

            def gates_lead_full(rhs):
                """Whole-bank injection: one K=128 identity matmul streams a
                precomputed [128=(c,b), 512] projection row into all four
                chunk positions at once."""
                bank = gps.tile([128, 512], F32, tag="g")
                nc.tensor.matmul(
                    bank, ident128_s, rhs, start=True, stop=False,
                    tile_position=(0, 0),
                )
                return bank

            def gates_lead(first_k):
                """Bank alloc + the NN state-independent leading matmuls of
                a step (x-projection / xp1 injection). Issued for BOTH layers
                before ANY hT-dependent matmul so the transpose->copy->sem
                chains drain under ~1.7us of independent PE work."""
                bank = gps.tile([128, 512], F32, tag="g")
                for n in range(NN):
                    lhsT0, rhs0, row0 = first_k(n)
                    nc.tensor.matmul(
                        bank[n * BL : (n + 1) * BL, :], lhsT0, rhs0,
                        start=True, stop=False,
                        tile_position=(row0, n * BL),
                    )
                return bank

            def gates_acc(bank, w_s, kslices):
                """The NN x len(kslices) recurrence accumulation matmuls."""
                for n in range(NN):
                    out = bank[n * BL : (n + 1) * BL, :]
                    for j, (lhsT_k, wk) in enumerate(kslices):
                        nc.tensor.matmul(
                            out,
                            lhsT_k,
                            w_s[:, wk, n * 512 : (n + 1) * 512],
                            start=False,
                            stop=(j == len(kslices) - 1),
                            tile_position=(0, n * BL),
                        )

            def chain(bank, cell, tag):
                """Nonlinearities + cell update, all bf16, all ops on
                [128,*] same-partition tiles. Bank partition block c holds
                hidden units [128c:128c+128); free layout per chunk is
                [i|f|o|g] x 128 (host-side column permutation). cell/h are
                [128=(c,b), 128=j] with j the unit within block c."""
                sig_ifo = work.tile([128, 384], BF16, tag=f"sig{tag}")
                nc.scalar.activation(sig_ifo, bank[:, 0:384], SIG)
                tanh_g = work.tile([128, 128], BF16, tag=f"tg{tag}")
                nc.scalar.activation(tanh_g, bank[:, 384:512], TANH)
                ig = work.tile([128, 128], BF16, tag=f"ig{tag}")
                cf = work.tile([128, 128], BF16, tag=f"cf{tag}")
                # ig on one engine, cf on the other, in parallel; the rest of
                # the serial tail on the (fast) DVE.
                nc.vector.tensor_mul(ig, sig_ifo[:, 0:128], tanh_g)
                nc.gpsimd.tensor_mul(cf, cell, sig_ifo[:, 128:256])
                nc.vector.tensor_add(cell, cf, ig)
                tanh_c = work.tile([128, 128], BF16, tag=f"tc{tag}")
                nc.scalar.activation(tanh_c, cell, TANH)
                h_new = work.tile([128, 128], BF16, tag=f"hn{tag}")
                nc.vector.tensor_mul(h_new, sig_ifo[:, 256:384], tanh_c)
                return h_new

            def transpose_to(h_new, dst):
                """h_new [128=(c,b), 128=j] -> dst [128, NK_H, 32] (hT) via a
                single [128,128] PE transpose + a single copy: h_new^T in
                (j, (c,b)) order IS hT chunk-concatenated."""
                tp = tpps.tile([128, 128], BF16, tag="tp")
                nc.tensor.transpose(tp, h_new, ident128_s)
                nc.vector.tensor_copy(
                    dst, tp.rearrange("p (a b) -> p a b", a=NK_H))

            h0_new = None  # h_new of layer0 step t-1 (awaiting transpose)
            h1_new = None  # h_new of layer1 step t'-1

            # Pipeline: iteration (b, s) runs layer0 step t=b*SB+s and
            # layer1 step t-SB. Issue order per pair keeps PE fed:
            #   tp0(t-1) -> L0mm(t) -> tp1(t'-1) -> L1mm(t') -> chains.
            # h0blk slot j holds h0 of step b*SB+j (written at iter s=j+1,
            # slot (s-1)%SB). The bulk GEMM for a 4-step group fires at the
            # next s%4==0 boundary, once all 4 slots are written.
            for b in range(nblk + 1):
                for s in range(SB):
                    t = b * SB + s
                    if h0_new is not None and (b < nblk or s == 0):
                        transpose_to(h0_new, h0blk[:, :, (s - 1) % SB, :])
                        h0_new = None
                    g0 = None
                    if b < nblk:
                        g0 = gates_lead(
                            lambda n, t=t: (
                                xt_s[:, t, :],
                                w0a_s[:, n * 512 : (n + 1) * 512],
                                0,
                            )
                        )
                    if g0 is not None:
                        prev = h0blk[:, :, (s - 1) % SB, :]
                        gates_acc(g0, w0b_s,
                                  [(prev[:, k, :], k) for k in range(NK_H)])
                    if b >= 1:
                        if h1_new is not None:
                            transpose_to(h1_new, h1T)
                            h1_new = None
                        g1 = gates_lead_full(xp1R[(b - 1) % 2][:, s, :])
                        gates_acc(g1, w1_s,
                                  [(h1T[:, k, :], NK_H + k) for k in range(NK_H)])
                    if b < nblk:
                        h0_new = chain(g0, c0, "0")
                    if b >= 1:
                        h1_new = chain(g1, c1, "1")
                    # bulk layer-1 input projection for the just-completed
                    # 4-step group (slots s-4..s-1 of block bb).
                    if s % 4 == 0 and t >= 4 and (b < nblk or s == 0):
                        bb, m = (b, s // 4 - 1) if s > 0 else (b - 1, SB // 4 - 1)
                        xpo = xp1R[bb % 2]
                        for n in range(NN):
                            ns = slice(n * 512, (n + 1) * 512)
                            xps = bkps.tile([128, 512], F32, tag="bk")
                            nc.tensor.matmul(
                                xps, ones_r128, w1b_s[:, ns],
                                start=True, stop=False,
                            )
                            for k in range(NK_H):
                                nc.tensor.matmul(
                                    xps,
                                    h0blk[:, k, 4 * m : 4 * m + 4, :].rearrange(
                                        "p a b -> p (a b)"
                                    ),
                                    w1_s[:, k, ns],
                                    start=False,
                                    stop=(k == NK_H - 1),
                                )
                            stg = work.tile([128, 512], BF16, tag="stg")
                            nc.vector.tensor_copy(stg[:, 0:256], xps[:, 0:256])
                            nc.vector.tensor_copy(stg[:, 256:512], xps[:, 256:512])
                            for si in range(4):
                                nc.sync.dma_start(
                                    out=xpo[n * BL : (n + 1) * BL, 4 * m + si, :],
                                    in_=stg[si * BL : (si + 1) * BL, :],
                                )

            # flush the last transposes (layer1's final h for fc)
            if h1_new is not None:
                transpose_to(h1_new, h1T)

            # --- fc on last h1 ---
            fcp = bkps.tile([BL, O], F32, tag="fc")
            for k in range(NK_H):
                nc.tensor.matmul(
                    fcp,
                    h1T[:, k, :],
                    fcw_s[:, k : k + 1],
                    start=(k == 0),
                    stop=(k == NK_H - 1),
                )
            y_s = work.tile([BL, O], F32, tag="y")
            nc.vector.tensor_add(y_s, fcp, fcb_s)
            nc.sync.dma_start(out=y_d[:, :], in_=y_s)

    _split_excess_waits(nc)
    return nc


# host-side gate-column permutation: torch gate order is (i,f,g,o); chunk c
# holds [i|f|o|g] x 128 for hidden units [128c:128c+128) so that sigmoid
# covers free cols 0:384 and tanh 384:512 across all partition blocks.
_PERM = np.concatenate(
    [np.concatenate(
        [np.arange(g * H + 128 * c, g * H + 128 * c + 128) for g in (0, 1, 3, 2)]
    ) for c in range(NK_H)]
)


def prep_inputs(x, w_ih_0, w_hh_0, b_ih_0, b_hh_0, w_ih_1, w_hh_1, b_ih_1, b_hh_1,
                fc_w, fc_b, t_steps: int = T):
    """Host-side layout prep + sharding. Returns per-core in_maps."""
    bf = ml_dtypes.bfloat16
    w0a = np.concatenate(
        [w_ih_0.T, (b_ih_0 + b_hh_0)[None, :]], axis=0
    )[:, _PERM].astype(bf)  # [65, G]
    w0b = np.ascontiguousarray(
        w_hh_0.T[:, _PERM].reshape(NK_H, 128, G).transpose(1, 0, 2)
    ).astype(bf)  # [128, 4, G]
    w1 = np.ascontiguousarray(
        np.concatenate([w_ih_1.T, w_hh_1.T], axis=0)[:, _PERM]
        .reshape(2 * NK_H, 128, G)
        .transpose(1, 0, 2)
    ).astype(bf)  # [128, 8, G]
    w1bias = (b_ih_1 + b_hh_1)[None, _PERM].astype(bf)  # [1, G]
    fcw = np.ascontiguousarray(fc_w.reshape(NK_H, 128).T).astype(bf)  # [128, 4]
    fcb = fc_b.reshape(1, 1).astype(np.float32)
    ident128 = np.eye(128, dtype=np.float32).astype(bf)

    in_maps = []
    for c in range(NCORES):
        xc = x[c * BL : (c + 1) * BL, :t_steps, :]  # [32, T, 64]
        xt = np.transpose(xc, (2, 1, 0))  # [64, T, 32]
        xt = np.concatenate([xt, np.ones((1, t_steps, BL), np.float32)], axis=0)
        in_maps.append(
            {
                "xt": np.ascontiguousarray(xt).astype(bf),
                "w0a": w0a,
                "w0b": w0b,
                "w1": w1,
                "w1bias": w1bias,
                "fcw": fcw,
                "fcb": fcb,
                "ident128": ident128,
            }
        )
    return in_maps


_NC_CACHE = {}


def kernel(x, w_ih_0, w_hh_0, b_ih_0, b_hh_0, w_ih_1, w_hh_1, b_ih_1, b_hh_1,
           fc_w, fc_b):
    x = np.asarray(x, np.float32)
    args = [np.asarray(a, np.float32) for a in (
        w_ih_0, w_hh_0, b_ih_0, b_hh_0, w_ih_1, w_hh_1, b_ih_1, b_hh_1, fc_w, fc_b)]
    if T not in _NC_CACHE:
        _NC_CACHE[T] = build_lstm_nc(T)
    nc = _NC_CACHE[T]
    in_maps = prep_inputs(x, *args, t_steps=T)
    res = run_bass_kernel_spmd(nc, in_maps, core_ids=list(range(NCORES)))
    return np.concatenate([res.results[c]["y"] for c in range(NCORES)], axis=0)
